# revision 4
# baseline (speedup 1.0000x reference)
"""Word-encoder masked-attention pooling (segment softmax-reduce) on 8 trn2 cores.

Strategy: shard the n_words dimension across 8 cores (750 words each).
Spans are contiguous and sorted, so each core only touches a contiguous
range of hidden_states rows.  The host packs, per core, the deduplicated
range of hidden rows as KB blocks of 128 rows in fp16 (each block carries
an extra "ones" column), plus 0/1 span masks in fp8 (exact).  Device:

    s      = H_blk . w              (DVE fused mul-reduce, fp16 in / f32 accum)
    E      = exp(s)                 (ACT, batched per DMA chunk)
    eh     = [H_blk * E | E]        (ACT copy-scale; ones column becomes E)
    num    = mask^T @ eh[:, :1024]  (PE, fp8 weights x fp16 moving, f32 PSUM)
    den    = mask^T @ eh[:, 1024]   (PE, same weights, N=1)
    out    = num * (1/den)          (DVE recip + GPSIMD scale, fp16 out)

which equals softmax(masked scores) @ hidden_states (b_attn cancels).
The host converts the fp16 output back to f32.  No cross-core comms.
"""

import numpy as np
from contextlib import ExitStack

import concourse.bass as bass
import concourse.bacc as bacc
import concourse.mybir as mybir
import concourse.tile as tile
from concourse.bass_utils import run_bass_kernel_spmd

NCORES = 8
P = 128
HID = 1024
BLK = HID + 1  # block width in hp: 1024 H cols + 1 ones col

LAST_RESULT = None  # BassKernelResults of the most recent run (for profiling)

_prog_cache = {}


def _chunk_blocks(kbc):
    """Split blocks 0..kbc-1 into DMA chunks of ~3 blocks."""
    chunks = []
    c0 = 0
    while c0 < kbc:
        nb = min(3, kbc - c0)
        chunks.append((c0, nb))
        c0 += nb
    return chunks


def _build_program(kbc, wins, bases):
    MT = len(wins)
    PAIRS = sum(wins)
    f32 = mybir.dt.float32
    f16 = mybir.dt.float16
    f8 = mybir.dt.float8e4
    nc = bacc.Bacc("TRN2", target_bir_lowering=False, debug=False, num_devices=NCORES)
    hp = nc.declare_dram_parameter("hp", [P, kbc * BLK], f16, isOutput=False)
    mk = nc.declare_dram_parameter("mk", [P, PAIRS * P], f8, isOutput=False)
    wb = nc.declare_dram_parameter("wb", [P, HID], f16, isOutput=False)
    outp = nc.declare_dram_parameter("outp", [P, MT * HID], f16, isOutput=True)

    chunks = _chunk_blocks(kbc)

    with tile.TileContext(nc) as tc, ExitStack() as ctx:
        wpool = ctx.enter_context(tc.tile_pool(name="w", bufs=1))
        hpool = ctx.enter_context(tc.tile_pool(name="h", bufs=len(chunks)))
        mpool = ctx.enter_context(tc.tile_pool(name="m", bufs=1))
        spool = ctx.enter_context(tc.tile_pool(name="s", bufs=2 * len(chunks)))
        prodpool = ctx.enter_context(tc.tile_pool(name="prod", bufs=2))
        ehpool = ctx.enter_context(tc.tile_pool(name="eh", bufs=kbc))
        rpool = ctx.enter_context(tc.tile_pool(name="r", bufs=2))
        opool = ctx.enter_context(tc.tile_pool(name="o", bufs=2))
        pnpool = ctx.enter_context(tc.tile_pool(name="pn", bufs=2, space="PSUM"))
        pdpool = ctx.enter_context(tc.tile_pool(name="pd", bufs=2, space="PSUM"))

        # masks first on the gpsimd queue (SWDGE ~2us fixed; needed by PE @~3us)
        mkt = mpool.tile([P, PAIRS * P], f8)
        nc.gpsimd.dma_start(mkt[:], mk[:, :])

        wt = wpool.tile([P, HID], f16)
        nc.sync.dma_start(wt[:], wb[:, :])

        htiles = []
        for c0, nb in chunks:
            h = hpool.tile([P, nb * BLK], f16)
            nc.sync.dma_start(h[:], hp[:, c0 * BLK : (c0 + nb) * BLK])
            htiles.append(h)

        # scores + exp + eh, pipelined per chunk
        ehs = [None] * kbc
        for ci, (c0, nb) in enumerate(chunks):
            h = htiles[ci]
            s = spool.tile([P, nb], f32)
            for j in range(nb):
                prod = prodpool.tile([P, HID], f16)
                nc.vector.scalar_tensor_tensor(
                    out=prod[:],
                    in0=h[:, j * BLK : j * BLK + HID],
                    scalar=1.0,
                    in1=wt[:],
                    op0=mybir.AluOpType.mult,
                    op1=mybir.AluOpType.mult,
                    accum_out=s[:, j : j + 1],
                )
            e = spool.tile([P, nb], f32)
            nc.scalar.activation(e[:], s[:], mybir.ActivationFunctionType.Exp)
            for j in range(nb):
                eh = ehpool.tile([P, BLK], f16)
                # SBUF-only scale on the Pool engine (it cannot touch PSUM)
                nc.gpsimd.tensor_scalar_mul(
                    eh[:], h[:, j * BLK : (j + 1) * BLK], e[:, j : j + 1]
                )
                ehs[c0 + j] = eh

        # per word-tile: accumulate num/den over the block window, normalize
        q = 0
        for m in range(MT):
            pn = pnpool.tile([P, HID], f32)
            pd = pdpool.tile([P, 1], f32)
            for j in range(wins[m]):
                eh = ehs[bases[m] + j]
                mks = mkt[:, q * P : (q + 1) * P]
                first, last = j == 0, j == wins[m] - 1
                nc.tensor.matmul(
                    pd[:], mks, eh[:, HID : HID + 1], start=first, stop=last
                )
                nc.tensor.matmul(
                    pn[:, 0:512], mks, eh[:, 0:512], start=first, stop=last
                )
                nc.tensor.matmul(
                    pn[:, 512:1024], mks, eh[:, 512:1024], start=first, stop=last
                )
                q += 1
            r = rpool.tile([P, 1], f32)
            nc.vector.reciprocal(r[:], pd[:])
            o = opool.tile([P, HID], f16)
            nc.scalar.activation(
                o[:], pn[:], mybir.ActivationFunctionType.Copy, scale=r[:]
            )
            nc.sync.dma_start(outp[:, m * HID : (m + 1) * HID], o[:])

    nc.compile()
    return nc


def kernel(hidden_states, word_starts, word_ends, w_attn, b_attn):
    global LAST_RESULT
    H = np.asarray(hidden_states, dtype=np.float32)
    ws = np.asarray(word_starts).astype(np.int64)
    we = np.asarray(word_ends).astype(np.int64)
    wv = np.asarray(w_attn, dtype=np.float32).reshape(-1)
    ns, hid = H.shape
    nw = ws.shape[0]
    assert hid == HID
    WPC = (nw + NCORES - 1) // NCORES  # words per core
    MT = (WPC + P - 1) // P  # word-tiles per core

    def tile_bounds(c, m):
        lo = c * WPC + m * P
        hi = min(lo + P, (c + 1) * WPC, nw)
        return lo, hi

    rstart = [int(ws[min(c * WPC, nw - 1)]) for c in range(NCORES)]
    # per (core, tile): block range relative to the core's row start
    blo = np.zeros((NCORES, MT), int)
    bhi = np.zeros((NCORES, MT), int)
    kb_core = []
    for c in range(NCORES):
        kbc = 1
        for m in range(MT):
            lo, hi = tile_bounds(c, m)
            if lo >= hi:
                blo[c, m] = bhi[c, m] = 0
                continue
            blo[c, m] = (int(ws[lo]) - rstart[c]) // P
            bhi[c, m] = (int(we[lo:hi].max()) - rstart[c]) // P
            kbc = max(kbc, int(bhi[c, m]) + 1)
        kb_core.append(kbc)
    bases = tuple(int(blo[:, m].min()) for m in range(MT))
    wins = tuple(
        int(bhi[:, m].max()) - bases[m] + 1 for m in range(MT)
    )
    kbc = max(max(kb_core), max(bases[m] + wins[m] for m in range(MT)))
    PAIRS = sum(wins)

    f16 = np.float16
    f8np = mybir.dt.np(mybir.dt.float8e4)
    wbp = np.ascontiguousarray(np.broadcast_to(wv[None, :], (P, HID))).astype(f16)
    in_maps = []
    for c in range(NCORES):
        # H blocks (+ ones column per block), fp16
        hp = np.zeros((P, kbc * BLK), f16)
        r0 = rstart[c]
        rows = min(kbc * P, ns - r0)
        hcols = H[r0 : r0 + rows].astype(f16)  # [rows, HID]
        for t in range(kbc):
            a, b = t * P, min((t + 1) * P, rows)
            if a < b:
                hp[: b - a, t * BLK : t * BLK + HID] = hcols[a:b]
            hp[:, t * BLK + HID] = 1.0
        # masks, fp8 (0/1 exact)
        mkp = np.zeros((P, PAIRS * P), f8np)
        qq = 0
        for m in range(MT):
            lo, hi = tile_bounds(c, m)
            for j in range(wins[m]):
                if lo < hi:
                    g0 = r0 + (bases[m] + j) * P  # first subword row of block
                    a = (ws[lo:hi] - g0).astype(np.int64)  # [words]
                    b = (we[lo:hi] - g0).astype(np.int64)
                    jj = np.arange(P, dtype=np.int64)[:, None]
                    mkp[:, qq * P : qq * P + (hi - lo)] = (
                        (jj >= a[None, :]) & (jj <= b[None, :])
                    ).astype(f8np)
                qq += 1
        in_maps.append({"hp": hp, "mk": mkp, "wb": wbp})

    key = (kbc, wins, bases)
    nc = _prog_cache.get(key)
    if nc is None:
        nc = _build_program(kbc, wins, bases)
        _prog_cache[key] = nc

    res = run_bass_kernel_spmd(nc, in_maps, list(range(NCORES)))
    LAST_RESULT = res
    full = np.empty((nw, HID), np.float32)
    for c in range(NCORES):
        op = np.asarray(res.results[c]["outp"], dtype=np.float32)  # [P, MT*HID]
        for m in range(MT):
            lo, hi = tile_bounds(c, m)
            if lo < hi:
                full[lo:hi] = op[: hi - lo, m * HID : (m + 1) * HID]
    return np.ascontiguousarray(full)


# revision 5
# speedup vs baseline: 4.1313x; 4.1313x over previous
"""Word-encoder masked-attention pooling (segment softmax-reduce) on 8 trn2 cores.

Strategy: shard the n_words dimension across 8 cores (750 words each).
Spans are contiguous and sorted, so each core only touches a contiguous
range of hidden_states rows.  The host packs, per core, the deduplicated
range of hidden rows as KB blocks of 128 rows in fp16, plus 0/1 span
masks in fp8 (exact), grouped by hidden-block.  Device:

    s      = H_blk . w              (DVE fused mul-reduce, fp16, f32 accum)
    E      = exp(s)                 (ACT, batched per DMA chunk)
    mke    = mask * E               (ACT copy-scale, grouped per block)
    num    = mke^T @ H_blk          (PE, fp16, f32 PSUM accum over window)
    den    = mke^T @ ones           (PE, N=1, same weights)
    out    = num * (1/den)          (DVE recip; scale split ACT/DVE, fp16 out)

which equals softmax(masked scores) @ hidden_states (b_attn cancels).
The host converts the fp16 output back to f32.  No cross-core comms.
"""

import numpy as np
from contextlib import ExitStack

import concourse.bass as bass
import concourse.bacc as bacc
import concourse.mybir as mybir
import concourse.tile as tile
from concourse.bass_utils import run_bass_kernel_spmd

NCORES = 8
P = 128
HID = 1024

LAST_RESULT = None  # BassKernelResults of the most recent run (for profiling)

_prog_cache = {}


def _chunk_blocks(kbc):
    """Split blocks 0..kbc-1 into DMA chunks of ~3 blocks."""
    chunks = []
    c0 = 0
    while c0 < kbc:
        nb = min(3, kbc - c0)
        chunks.append((c0, nb))
        c0 += nb
    return chunks


def _groups(kbc, wins, bases):
    """Pairs (m, j) grouped by the hidden block t they consume.
    Returns groups[t] = [m...] and col_of[(m, j)] = column offset in the
    per-block mask slab."""
    groups = [[] for _ in range(kbc)]
    col_of = {}
    for m in range(len(wins)):
        for j in range(wins[m]):
            t = bases[m] + j
            col_of[(m, j)] = len(groups[t])
            groups[t].append(m)
    return groups, col_of


def _build_program(kbc, wins, bases):
    MT = len(wins)
    PAIRS = sum(wins)
    groups, col_of = _groups(kbc, wins, bases)
    goff = [0] * kbc  # column-group start of block t in the mask slab
    for t in range(1, kbc):
        goff[t] = goff[t - 1] + len(groups[t - 1])

    f32 = mybir.dt.float32
    f16 = mybir.dt.float16
    f8 = mybir.dt.float8e4
    nc = bacc.Bacc("TRN2", target_bir_lowering=False, debug=False, num_devices=NCORES)
    hp = nc.declare_dram_parameter("hp", [P, kbc * HID], f16, isOutput=False)
    mk = nc.declare_dram_parameter("mk", [P, PAIRS * P], f8, isOutput=False)
    wb = nc.declare_dram_parameter("wb", [P, HID], f16, isOutput=False)
    outp = nc.declare_dram_parameter("outp", [P, MT * HID], f16, isOutput=True)

    chunks = _chunk_blocks(kbc)

    with tile.TileContext(nc) as tc, ExitStack() as ctx:
        wpool = ctx.enter_context(tc.tile_pool(name="w", bufs=1))
        hpool = ctx.enter_context(tc.tile_pool(name="h", bufs=len(chunks)))
        mpool = ctx.enter_context(tc.tile_pool(name="m", bufs=1))
        spool = ctx.enter_context(tc.tile_pool(name="s", bufs=2 * len(chunks)))
        prodpool = ctx.enter_context(tc.tile_pool(name="prod", bufs=2))
        mkepool = ctx.enter_context(tc.tile_pool(name="mke", bufs=kbc))
        rpool = ctx.enter_context(tc.tile_pool(name="r", bufs=2))
        opool = ctx.enter_context(tc.tile_pool(name="o", bufs=3))
        pnpool = ctx.enter_context(tc.tile_pool(name="pn", bufs=2, space="PSUM"))
        pdpool = ctx.enter_context(tc.tile_pool(name="pd", bufs=2, space="PSUM"))
        wupool = ctx.enter_context(tc.tile_pool(name="wu", bufs=1))
        wupsum = ctx.enter_context(tc.tile_pool(name="wup", bufs=1, space="PSUM"))

        # PE warm-up: dummy matmuls starting immediately so HAM reaches
        # K=8/8 (2.4 GHz) before the real accumulation pairs arrive.
        wu = wupool.tile([P, P], f16)
        nc.vector.memset(wu[:], 0.0)
        ones = wupool.tile([P, 1], f16)
        nc.vector.memset(ones[:], 1.0)
        wups = wupsum.tile([P, P], f32)
        for _ in range(28):
            nc.tensor.matmul(wups[:], wu[:], wu[:], start=True, stop=True)

        # masks on the gpsimd queue (SWDGE ~2us fixed; needed by PE @~4us)
        mkt = mpool.tile([P, PAIRS * P], f8)
        nc.gpsimd.dma_start(mkt[:], mk[:, :])

        wt = wpool.tile([P, HID], f16)
        nc.sync.dma_start(wt[:], wb[:, :])

        htiles = []
        for c0, nb in chunks:
            h = hpool.tile([P, nb * HID], f16)
            nc.sync.dma_start(h[:], hp[:, c0 * HID : (c0 + nb) * HID])
            htiles.append(h)

        def hblk(t, cs=slice(0, HID)):
            for ci, (c0, nb) in enumerate(chunks):
                if c0 <= t < c0 + nb:
                    base = (t - c0) * HID
                    return htiles[ci][:, base + cs.start : base + cs.stop]
            raise AssertionError(t)

        # scores + exp per chunk, then per-block mask scaling
        mkes = [None] * kbc
        for ci, (c0, nb) in enumerate(chunks):
            h = htiles[ci]
            s = spool.tile([P, nb], f32)
            for j in range(nb):
                prod = prodpool.tile([P, HID], f16)
                nc.vector.scalar_tensor_tensor(
                    out=prod[:],
                    in0=h[:, j * HID : (j + 1) * HID],
                    scalar=1.0,
                    in1=wt[:],
                    op0=mybir.AluOpType.mult,
                    op1=mybir.AluOpType.mult,
                    accum_out=s[:, j : j + 1],
                )
            e = spool.tile([P, nb], f32)
            nc.scalar.activation(e[:], s[:], mybir.ActivationFunctionType.Exp)
            for j in range(nb):
                t = c0 + j
                cnt = len(groups[t])
                if cnt == 0:
                    continue
                mke = mkepool.tile([P, cnt * P], f16)
                nc.scalar.activation(
                    mke[:],
                    mkt[:, goff[t] * P : (goff[t] + cnt) * P],
                    mybir.ActivationFunctionType.Copy,
                    scale=e[:, j : j + 1],
                )
                mkes[t] = mke

        # per word-tile: accumulate num/den over the block window, normalize
        for m in range(MT):
            pn = pnpool.tile([P, HID], f32)
            pd = pdpool.tile([P, 1], f32)
            for j in range(wins[m]):
                t = bases[m] + j
                c = col_of[(m, j)]
                mks = mkes[t][:, c * P : (c + 1) * P]
                first, last = j == 0, j == wins[m] - 1
                nc.tensor.matmul(pd[:], mks, ones[:], start=first, stop=last)
                nc.tensor.matmul(
                    pn[:, 0:512], mks, hblk(t, slice(0, 512)), start=first, stop=last
                )
                nc.tensor.matmul(
                    pn[:, 512:1024],
                    mks,
                    hblk(t, slice(512, 1024)),
                    start=first,
                    stop=last,
                )
            r = rpool.tile([P, 1], f32)
            nc.vector.reciprocal(r[:], pd[:])
            o = opool.tile([P, HID], f16)
            if m % 2 == 0:
                nc.scalar.activation(
                    o[:], pn[:], mybir.ActivationFunctionType.Copy, scale=r[:]
                )
            else:
                nc.vector.tensor_scalar_mul(o[:], pn[:], r[:])
            nc.sync.dma_start(outp[:, m * HID : (m + 1) * HID], o[:])

    nc.compile()
    return nc


def kernel(hidden_states, word_starts, word_ends, w_attn, b_attn):
    global LAST_RESULT
    H = np.asarray(hidden_states, dtype=np.float32)
    ws = np.asarray(word_starts).astype(np.int64)
    we = np.asarray(word_ends).astype(np.int64)
    wv = np.asarray(w_attn, dtype=np.float32).reshape(-1)
    ns, hid = H.shape
    nw = ws.shape[0]
    assert hid == HID
    WPC = (nw + NCORES - 1) // NCORES  # words per core
    MT = (WPC + P - 1) // P  # word-tiles per core

    def tile_bounds(c, m):
        lo = c * WPC + m * P
        hi = min(lo + P, (c + 1) * WPC, nw)
        return lo, hi

    rstart = [int(ws[min(c * WPC, nw - 1)]) for c in range(NCORES)]
    blo = np.zeros((NCORES, MT), int)
    bhi = np.zeros((NCORES, MT), int)
    kb_core = []
    for c in range(NCORES):
        kbc = 1
        for m in range(MT):
            lo, hi = tile_bounds(c, m)
            if lo >= hi:
                blo[c, m] = bhi[c, m] = 0
                continue
            blo[c, m] = (int(ws[lo]) - rstart[c]) // P
            bhi[c, m] = (int(we[lo:hi].max()) - rstart[c]) // P
            kbc = max(kbc, int(bhi[c, m]) + 1)
        kb_core.append(kbc)
    bases = tuple(int(blo[:, m].min()) for m in range(MT))
    wins = tuple(int(bhi[:, m].max()) - bases[m] + 1 for m in range(MT))
    kbc = max(max(kb_core), max(bases[m] + wins[m] for m in range(MT)))
    PAIRS = sum(wins)
    groups, col_of = _groups(kbc, wins, bases)

    f16 = np.float16
    f8np = mybir.dt.np(mybir.dt.float8e4)
    wbp = np.ascontiguousarray(np.broadcast_to(wv[None, :], (P, HID))).astype(f16)
    in_maps = []
    for c in range(NCORES):
        # H blocks, fp16
        hp = np.zeros((P, kbc * HID), f16)
        r0 = rstart[c]
        rows = min(kbc * P, ns - r0)
        hrows = H[r0 : r0 + rows].astype(f16)  # [rows, HID]
        for t in range(kbc):
            a, b = t * P, min((t + 1) * P, rows)
            if a < b:
                hp[: b - a, t * HID : (t + 1) * HID] = hrows[a:b]
        # masks, fp8 (0/1 exact), grouped by block
        mkp = np.zeros((P, PAIRS * P), f8np)
        qq = 0
        for t in range(kbc):
            for m in groups[t]:
                j = t - bases[m]
                lo, hi = tile_bounds(c, m)
                if lo < hi:
                    g0 = r0 + t * P  # first subword row of block t
                    a = (ws[lo:hi] - g0).astype(np.int64)
                    b = (we[lo:hi] - g0).astype(np.int64)
                    jj = np.arange(P, dtype=np.int64)[:, None]
                    mkp[:, qq * P : qq * P + (hi - lo)] = (
                        (jj >= a[None, :]) & (jj <= b[None, :])
                    ).astype(f8np)
                qq += 1
        in_maps.append({"hp": hp, "mk": mkp, "wb": wbp})

    key = (kbc, wins, bases)
    nc = _prog_cache.get(key)
    if nc is None:
        nc = _build_program(kbc, wins, bases)
        _prog_cache[key] = nc

    res = run_bass_kernel_spmd(nc, in_maps, list(range(NCORES)))
    LAST_RESULT = res
    full = np.empty((nw, HID), np.float32)
    for c in range(NCORES):
        op = np.asarray(res.results[c]["outp"], dtype=np.float32)  # [P, MT*HID]
        for m in range(MT):
            lo, hi = tile_bounds(c, m)
            if lo < hi:
                full[lo:hi] = op[: hi - lo, m * HID : (m + 1) * HID]
    return np.ascontiguousarray(full)
